# revision 11
# baseline (speedup 1.0000x reference)
"""Trainium2 Bass kernel for BackgroundSubtractorModule.

Reference computation (per 15-frame window, gray video):
  y      = 0.299 R + 0.587 G + 0.114 B            (per pixel, x scale)
  m      = mean_f y ; var = sum_f (y-m)^2 / 14
  sigma  = sqrt(var) + 1e-5
  bg     = |y - m| / sigma
  minv/maxv = min/max over pixels of bg (per frame)
  out    = (bg - minv) / (maxv - minv)  if rng > 1e-6 else bg

Sharding: 30 independent windows across 8 cores; every core runs an
identical 4-window program (cores 6,7 process one duplicated pad window
whose output is dropped).

Implementation notes (HW-measured rates drove the design):
  * Scaled luma: y' = (w0/w1) R + G + (w2/w1) B; the w1 factor is folded
    into the sigma scale and into inv_sigma, so G needs no scaled copy.
  * Channel extraction (stride-3 reads) runs on DVE tensor_scalar (2x
    mode, 0.93us/frame) and ACT scaled copies, split by knob.
  * Frame sum and sum-of-squares accumulate on the otherwise-idle PE as
    identity-matmul PSUM accumulations (bit-exact f32, bank-aligned
    512/512/128 column slices).
  * abs and the per-frame min/max reduces run chunked over 3-frame
    groups (FD 3456) to amortize per-instruction overhead.
  * Normalize is one ACT op per frame: Identity(bg*inv_rng + c) with
    per-partition scale/bias vectors, written in place over the window
    buffer; one 8.8MB DMA stores the window.
  * Cross-partition min/max via GPSIMD partition_all_reduce(max) on
    [maxv | -minv].
"""

import numpy as np
from contextlib import ExitStack

import concourse.bass as bass
import concourse.bacc as bacc
import concourse.tile as tile
from concourse import mybir, bass_isa
from concourse.bass_utils import run_bass_kernel_spmd

F32 = mybir.dt.float32
OP = mybir.AluOpType
AF = mybir.ActivationFunctionType

T, H, W = 450, 384, 384
PIX = H * W                    # 147456
WIN = 15
NCORES = 8
NWIN_CORE = 4                  # ceil(30/8) -> uniform SPMD program
FPC = NWIN_CORE * WIN          # 60 frames per core
P = 128
COLS = PIX // P                # 1152
EPS = 1e-5
THRESH = 1e-6
BANKS = ((0, 512), (512, 1024), (1024, 1152))   # PSUM bank-aligned slices

# engine-balance knobs
N_COPY_ACT = 14        # frames whose two scaled channel copies run on ACT
N_MULT_DVE = 6         # frames whose bg multiply runs on DVE (rest GPSIMD)
N_SUB_DVE = 15         # frames whose d=y-m runs on DVE (rest GPSIMD)

_BUILD_CACHE = {}


def _build(scale: float):
    w0, w1, w2 = 0.299 * scale, 0.587 * scale, 0.114 * scale
    a_r, a_b = w0 / w1, w2 / w1
    nc = bacc.Bacc("TRN2", target_bir_lowering=False, debug=False)
    vin = nc.dram_tensor("video", [FPC, PIX * 3], F32, kind="ExternalInput").ap()
    idd = nc.dram_tensor("ident", [P, P], F32, kind="ExternalInput").ap()
    vout = nc.dram_tensor("out", [FPC, PIX], F32, kind="ExternalOutput").ap()

    with tile.TileContext(nc) as tc, ExitStack() as ctx:
        p_const = ctx.enter_context(tc.tile_pool(name="const", bufs=1))
        p_y = ctx.enter_context(tc.tile_pool(name="y", bufs=2))
        p_rgb = ctx.enter_context(tc.tile_pool(name="rgb", bufs=2))
        p_stat = ctx.enter_context(tc.tile_pool(name="stat", bufs=2))
        p_tmp = ctx.enter_context(tc.tile_pool(name="tmp", bufs=5))
        p_mm = ctx.enter_context(tc.tile_pool(name="mm", bufs=2))
        p_ps = ctx.enter_context(tc.tile_pool(name="psum", bufs=1, space="PSUM"))

        ident = p_const.tile([P, P], F32)
        nc.sync.dma_start(ident[:], idd[:])

        for w in range(NWIN_CORE):
            yt = p_y.tile([P, WIN * COLS], F32, tag="y")
            acc_s = p_ps.tile([P, COLS], F32, tag="acc_s")    # sum_f y'
            mt = p_stat.tile([P, COLS], F32, tag="m")         # mean
            st = p_stat.tile([P, COLS], F32, tag="s")         # ssq->var->sigma->inv
            mmt = p_mm.tile([P, 96], F32, tag="mm")
            # mmt cols: 0:16 max(bg) pp -> later c1, 16:32 min(bg) pp (negated),
            #           32:48 allred max, 48:64 allred -min, 64:80 rng->inv_rng,
            #           80:96 mask
            nc.gpsimd.memset(mmt[:], 0.0)  # cols 15,31 have no frame

            def yslice(f):
                return yt[:, f * COLS:(f + 1) * COLS]

            # ---- P1: luma + PE-accumulated sum / sum of squares ----
            for f in range(WIN):
                g = w * WIN + f
                rgbt = p_rgb.tile([P, COLS * 3], F32, tag="rgb")
                nc.sync.dma_start(rgbt[:], vin[g].rearrange("(r j) -> r j", r=P))
                rgb3 = rgbt[:].rearrange("p (j c) -> p j c", c=3)
                yf = yslice(f)
                t2 = p_tmp.tile([P, COLS], F32, tag="tmp")
                if f < N_COPY_ACT:
                    nc.scalar.activation(yf, rgb3[:, :, 0], AF.Copy, bias=0.0, scale=a_r)
                    nc.scalar.activation(t2[:], rgb3[:, :, 2], AF.Copy, bias=0.0, scale=a_b)
                else:
                    nc.vector.tensor_scalar(yf, rgb3[:, :, 0], a_r, None, OP.mult)
                    nc.vector.tensor_scalar(t2[:], rgb3[:, :, 2], a_b, None, OP.mult)
                nc.gpsimd.tensor_tensor(yf, yf, rgb3[:, :, 1], OP.add)   # += G
                nc.gpsimd.tensor_tensor(yf, yf, t2[:], OP.add)
                sq = p_tmp.tile([P, COLS], F32, tag="tmp")
                nc.scalar.activation(sq[:], yf, AF.Square)
                if f == 0:
                    nc.vector.tensor_copy(st[:], sq[:])
                else:
                    nc.vector.tensor_tensor(st[:], st[:], sq[:], OP.add)
                for lo, hi in BANKS:
                    nc.tensor.matmul(acc_s[:, lo:hi], ident[:], yf[:, lo:hi],
                                     start=(f == 0), stop=(f == WIN - 1))

            # ---- P2: mean / inv_sigma (all on the scaled-luma domain) ----
            nc.vector.tensor_scalar(mt[:], acc_s[:], 1.0 / WIN, None, OP.mult)
            msq = p_tmp.tile([P, COLS], F32, tag="tmp")
            # 15*m^2 = Square(m * sqrt(15))
            nc.scalar.activation(msq[:], mt[:], AF.Square, scale=float(np.sqrt(15.0)))
            nc.vector.tensor_tensor(st[:], st[:], msq[:], OP.subtract)
            # true sigma = Sqrt(var' * w1^2 / 14)
            nc.scalar.activation(st[:], st[:], AF.Sqrt, scale=w1 * w1 / (WIN - 1))
            # recip input: (sigma + eps)/w1  ->  recip = w1/(sigma+eps)
            nc.vector.tensor_scalar(st[:], st[:], EPS, 1.0 / w1, OP.add, OP.mult)
            nc.vector.reciprocal(st[:], st[:])          # inv_sigma (w1-folded)

            # ---- P3: bg = |y'-m'| * inv_sigma', per-frame min/max ----
            for grp in range(WIN // 3):
                f0 = grp * 3
                for f in range(f0, f0 + 3):
                    eng = nc.vector if f < N_SUB_DVE else nc.gpsimd
                    eng.tensor_tensor(yslice(f), yslice(f), mt[:], OP.subtract)
                ych = yt[:, f0 * COLS:(f0 + 3) * COLS]
                nc.scalar.activation(ych, ych, AF.Abs)
                for f in range(f0, f0 + 3):
                    eng = nc.vector if f < N_MULT_DVE else nc.gpsimd
                    eng.tensor_tensor(yslice(f), yslice(f), st[:], OP.mult)
                ych3 = ych.rearrange("p (f j) -> p f j", f=3)
                nc.vector.tensor_reduce(
                    mmt[:, f0:f0 + 3], ych3, axis=mybir.AxisListType.X, op=OP.max)
                nc.vector.tensor_reduce(
                    mmt[:, 16 + f0:19 + f0], ych3, axis=mybir.AxisListType.X, op=OP.min)

            # ---- P4: cross-partition reduce + per-frame constants ----
            nc.vector.tensor_scalar(mmt[:, 16:32], mmt[:, 16:32], -1.0, None, OP.mult)
            nc.gpsimd.partition_all_reduce(
                mmt[:, 32:64], mmt[:, 0:32], 128, bass_isa.ReduceOp.max
            )
            mx = mmt[:, 32:48]      # maxv
            nmn = mmt[:, 48:64]     # -minv
            rng = mmt[:, 64:80]
            msk = mmt[:, 80:96]
            nc.vector.tensor_tensor(rng, mx, nmn, OP.add)            # maxv - minv
            nc.vector.tensor_scalar(msk, rng, THRESH, None, OP.is_gt)
            nc.vector.tensor_tensor(rng, rng, msk, OP.mult)
            nc.vector.tensor_scalar(rng, rng, 1.0, None, OP.add)
            nc.vector.tensor_tensor(rng, rng, msk, OP.subtract)      # rng_safe
            nc.vector.reciprocal(rng, rng)                           # inv_rng
            c1 = mmt[:, 0:16]
            nc.vector.tensor_tensor(c1, nmn, msk, OP.mult)           # -minv_eff
            nc.vector.tensor_tensor(c1, c1, rng, OP.mult)            # -minv_eff*inv_rng

            # ---- P5: out = bg*inv_rng + c, in place; 3-frame-group stores on
            # the scalar HWDGE queue so they interleave with next-window loads
            for grp in range(WIN // 3):
                f0 = grp * 3
                for f in range(f0, f0 + 3):
                    nc.scalar.activation(
                        yslice(f), yslice(f), AF.Identity,
                        bias=c1[:, f:f + 1], scale=rng[:, f:f + 1]
                    )
                g0 = w * WIN + f0
                nc.scalar.dma_start(
                    vout[g0:g0 + 3].rearrange("f (r j) -> r f j", r=P),
                    yt[:, f0 * COLS:(f0 + 3) * COLS].rearrange("p (f j) -> p f j", f=3),
                )

    nc.compile()
    return nc


def _get_nc(scale: float):
    key = round(float(scale), 9)
    if key not in _BUILD_CACHE:
        _BUILD_CACHE[key] = _build(key)
    return _BUILD_CACHE[key]


def kernel(video: np.ndarray) -> np.ndarray:
    video = np.ascontiguousarray(np.asarray(video, dtype=np.float32))
    assert video.shape == (T, H, W, 3), video.shape
    scale = 1.0 / 255.0 if float(video.max()) > 1.0 else 1.0
    nc = _get_nc(scale)

    v = video.reshape(T, PIX * 3)
    shards = []
    for c in range(6):
        shards.append(v[c * FPC:(c + 1) * FPC])
    # cores 6,7: 3 real windows + last window repeated as pad
    shards.append(np.concatenate([v[360:405], v[390:405]], axis=0))
    shards.append(np.concatenate([v[405:450], v[435:450]], axis=0))

    ident = np.eye(P, dtype=np.float32)
    res = run_bass_kernel_spmd(
        nc, [{"video": s, "ident": ident} for s in shards], list(range(NCORES))
    )
    outs = [res.results[c]["out"] for c in range(NCORES)]
    full = np.concatenate(
        [o[:FPC] for o in outs[:6]] + [outs[6][:45], outs[7][:45]], axis=0
    )
    return full.reshape(T, 1, H, W)
